# revision 2
# baseline (speedup 1.0000x reference)
"""Trainium2 Bass kernel for fused BERT-CRF-NER word_embedding + sigmoid.

Math (per batch row):
  inner[t]   = 1 <= t <= L-2          (L = valid length from contiguous mask)
  starts     = first_label_mask & inner
  word_id[t] = cumsum(starts) - 1     (-1 outside inner)
  wv[k]      = mean of token_features[t] over word_id[t] == k
  emission   = sigmoid(wv @ W.T + b)  (empty word slots -> sigmoid(b))

Key restructuring for the hardware: mean-pooling and the Linear layer are both
linear, so instead of segment-meaning 768-dim vectors we
  1) build the membership matrix M[t, k] = (word_id[t] == k)   [S, K]
  2) Z^T[d, k]   = sum_t X[t, d] * M[t, k]    (PE, X chunks as stationary
     operand in natural [t, d] layout -> no transpose of X is ever needed)
  3) logits^T[l, k] = sum_d W^T[d, l] * Z^T[d, k],  counts[k] = sum_t M[t, k]
  4) divide by counts (per-partition scalar after a tiny transpose), + bias,
     sigmoid, one output DMA per row.

The heavy matmuls run in bf16 (separate LDWEIGHTS overlaps with the matmul
stream; fp32 weights self-load and serialize). X is cast fp32->bf16 during the
DMA (SWDGE); accumulation stays fp32 in PSUM.
Sharding: pure data parallel, 8 batch rows per core across 8 cores.
"""

from contextlib import ExitStack

import numpy as np

import concourse.bass as bass
import concourse.tile as tile
from concourse import bacc, mybir
from concourse.bass_utils import run_bass_kernel_spmd

B, S, D, NL = 64, 512, 768, 10
N_CORES = 8
RPC = B // N_CORES  # batch rows per core
K = 224             # word-slot capacity (max words/row in data is 206)
TC = S // 128       # token chunks of 128
DC = D // 128       # feature chunks of 128
KC = 2              # k chunks (128 + 96)

f32 = mybir.dt.float32
bf16 = mybir.dt.bfloat16
i32 = mybir.dt.int32
Alu = mybir.AluOpType
Act = mybir.ActivationFunctionType


def _build_nc():
    nc = bacc.Bacc("TRN2", target_bir_lowering=False, debug=False)
    x_d = nc.dram_tensor("x", [RPC, S, D], f32, kind="ExternalInput")
    im_d = nc.dram_tensor("im", [RPC, S], i32, kind="ExternalInput")
    fm_d = nc.dram_tensor("fm", [RPC, S], i32, kind="ExternalInput")
    wt_d = nc.dram_tensor("wt", [D, NL], f32, kind="ExternalInput")
    b_d = nc.dram_tensor("b", [1, NL], f32, kind="ExternalInput")
    out_d = nc.dram_tensor("out", [RPC, S, NL], f32, kind="ExternalOutput")

    with tile.TileContext(nc) as tc, ExitStack() as ctx:
        const = ctx.enter_context(tc.tile_pool(name="const", bufs=1))
        xp = ctx.enter_context(tc.tile_pool(name="xp", bufs=8))
        mp = ctx.enter_context(tc.tile_pool(name="mp", bufs=3))
        zsp = ctx.enter_context(tc.tile_pool(name="zsp", bufs=3))
        rsp = ctx.enter_context(tc.tile_pool(name="rsp", bufs=2))
        obp = ctx.enter_context(tc.tile_pool(name="obp", bufs=2))
        ztp = ctx.enter_context(
            tc.tile_pool(name="ztp", bufs=2, space=bass.MemorySpace.PSUM)
        )
        lgp = ctx.enter_context(
            tc.tile_pool(name="lgp", bufs=1, space=bass.MemorySpace.PSUM)
        )
        ctp = ctx.enter_context(
            tc.tile_pool(name="ctp", bufs=1, space=bass.MemorySpace.PSUM)
        )
        tpp = ctx.enter_context(
            tc.tile_pool(name="tpp", bufs=2, space=bass.MemorySpace.PSUM)
        )

        # ---- X loads first: they pace the whole kernel -----------------
        # SWDGE cast-DMA fp32 -> bf16; tokens laid out t = 4p + c so each
        # partition line is one contiguous 12KB DRAM read. Row 0 issues
        # before any other Q7 work; the iotas ride between rows while the
        # SDMA engines drain. The Q7 stalls at row 5 on pool-slot reuse,
        # which is fine — it has no other work after the iotas.
        x_ts = []
        x_t0 = xp.tile([128, TC, D], bf16)
        xsrc0 = x_d[0].rearrange("(p c) d -> p c d", p=128)
        # row 0 in d-halves: stage-1 h=0 only needs d 0:384, so PE starts
        # as soon as the first half lands
        nc.gpsimd.dma_start(x_t0[:, :, 0 : D // 2], xsrc0[:, :, 0 : D // 2])
        nc.gpsimd.dma_start(x_t0[:, :, D // 2 : D], xsrc0[:, :, D // 2 : D])
        x_ts.append(x_t0)

        iota_ki = const.tile([128, K], i32)
        nc.gpsimd.iota(iota_ki[:], pattern=[[1, K]], base=0, channel_multiplier=0)
        iota_pi = const.tile([128, 1], i32)
        nc.gpsimd.iota(iota_pi[:], pattern=[[1, 1]], base=0, channel_multiplier=1)
        pos_i = const.tile([RPC, S], i32)
        nc.gpsimd.iota(pos_i[:], pattern=[[1, S]], base=0, channel_multiplier=0)
        for r in range(1, RPC):
            x_t = xp.tile([128, TC, D], bf16)
            nc.gpsimd.dma_start(x_t[:], x_d[r].rearrange("(p c) d -> p c d", p=128))
            x_ts.append(x_t)

        wt_f = const.tile([128, DC, NL], f32)
        nc.sync.dma_start(wt_f[:], wt_d.rearrange("(j p) l -> p j l", p=128))
        b_sb = const.tile([1, NL], f32)
        nc.sync.dma_start(b_sb[:], b_d[:, :])

        # ---- mask pipeline: word ids per token, all RPC rows at once ----
        im_i = const.tile([RPC, S], i32)
        nc.sync.dma_start(im_i[:], im_d[:, :])
        fm_i = const.tile([RPC, S], i32)
        nc.sync.dma_start(fm_i[:], fm_d[:, :])
        imf = const.tile([RPC, S], f32)
        nc.vector.tensor_copy(imf[:], im_i[:])
        fmf = const.tile([RPC, S], f32)
        nc.vector.tensor_copy(fmf[:], fm_i[:])

        L8 = const.tile([RPC, 1], f32)
        nc.vector.tensor_reduce(L8[:], imf[:], axis=mybir.AxisListType.X, op=Alu.add)
        lm2 = const.tile([RPC, 1], f32)
        nc.vector.tensor_scalar_add(lm2[:], L8[:], -2.0)

        posf = const.tile([RPC, S], f32)
        nc.vector.tensor_copy(posf[:], pos_i[:])

        inner = const.tile([RPC, S], f32)
        nc.vector.tensor_scalar(
            inner[:], posf[:], lm2[:, 0:1], None, op0=Alu.is_le
        )
        nc.vector.memset(inner[:, 0:1], 0.0)  # position 0 ([CLS]) excluded

        starts = const.tile([RPC, S], f32)
        nc.vector.tensor_mul(starts[:], fmf[:], inner[:])
        widr = const.tile([RPC, S], f32)
        nc.vector.tensor_tensor_scan(
            widr[:], starts[:], starts[:], 0.0, op0=Alu.add, op1=Alu.bypass
        )
        wid = const.tile([RPC, S], f32)
        nc.vector.tensor_mul(wid[:], widr[:], inner[:])
        nc.vector.tensor_scalar_add(wid[:], wid[:], -1.0)

        # DVE consts needed from here on (kept off the mask-chain critical path)
        iota_pf = const.tile([128, 1], f32)
        nc.vector.tensor_copy(iota_pf[:], iota_pi[:])
        iota_kf = const.tile([128, K], bf16)   # 0..223 exact in bf16
        nc.vector.tensor_copy(iota_kf[:], iota_ki[:])
        ident = const.tile([128, 128], f32)
        nc.vector.tensor_scalar(
            ident[:], iota_kf[:, 0:128], iota_pf[:, 0:1], None, op0=Alu.is_equal
        )
        ones_r = const.tile([128, 1], bf16)
        nc.vector.memset(ones_r[:], 1.0)
        ones1 = const.tile([1, 128], f32)
        nc.vector.memset(ones1[:], 1.0)
        wt = const.tile([128, DC, NL], bf16)
        nc.vector.tensor_copy(wt[:], wt_f[:])

        # sigmoid(b) broadcast [128, 2, NL] for the constant slot region
        sigb_row = const.tile([1, NL], f32)
        nc.scalar.activation(sigb_row[:], b_sb[:], Act.Sigmoid)
        sigb_ps = tpp.tile([128, 16], f32, tag="tp")
        nc.tensor.matmul(sigb_ps[:, 0:NL], ones1[0:1, :], sigb_row[0:1, :])
        sigb2 = const.tile([128, 2, NL], f32)
        nc.scalar.copy(sigb2[:, 0, :], sigb_ps[:, 0:NL])
        nc.scalar.copy(sigb2[:, 1, :], sigb_ps[:, 0:NL])

        # raw bias broadcast [128, 2, NL] (added along free dim post-transpose)
        bb_ps = tpp.tile([128, 16], f32, tag="tp")
        nc.tensor.matmul(bb_ps[:, 0:NL], ones1[0:1, :], b_sb[0:1, :])
        b_bc2 = const.tile([128, 2, NL], f32)
        nc.scalar.copy(b_bc2[:, 0, :], bb_ps[:, 0:NL])
        nc.scalar.copy(b_bc2[:, 1, :], bb_ps[:, 0:NL])

        # transpose word ids onto token partitions, in the t = 4p + c
        # interleaved order that matches the X tile layout below
        widT = const.tile([128, TC, RPC], f32)
        wid_v = wid[:].rearrange("r (p c) -> r p c", c=TC)
        for c in range(TC):
            tp_ps = tpp.tile([128, 16], f32, tag="tp")
            nc.tensor.transpose(
                tp_ps[:, 0:RPC], wid_v[:, :, c], ident[0:RPC, 0:RPC]
            )
            nc.vector.tensor_copy(widT[:, c, :], tp_ps[:, 0:RPC])

        countsT = const.tile([128, RPC, KC], f32)
        nc.vector.memset(countsT[:], 0.0)  # chunk 1 covers only 96 slots
        recipT = const.tile([128, RPC, KC], f32)

        # ---- heavy per-row pipeline, software-pipelined ----------------
        # PE executes in program order, so row r's stage-2 (which waits on
        # the Z^T PSUM->SBUF copies) is emitted AFTER row r+1's stage-1
        # matmuls: the copy latency hides under the next row's MM stream.
        zs_ts = {}

        def stage1(r):
            x_t = x_ts[r]
            m_t = mp.tile([128, TC, K], bf16, tag="m")
            for c in range(TC):
                nc.vector.tensor_scalar(
                    m_t[:, c, :], iota_kf[:], widT[:, c, r : r + 1], None,
                    op0=Alu.is_equal,
                )
            zs_t = zsp.tile([128, DC, K], bf16, tag="zs")
            for h in range(2):
                # per-j stride padded to 256 so each slice stays in one bank
                zt_ps = ztp.tile([128, DC // 2, 256], f32, tag="zt")
                for jj in range(DC // 2):
                    j = h * (DC // 2) + jj
                    for c in range(TC):
                        nc.tensor.matmul(
                            zt_ps[:, jj, 0:K],
                            x_t[:, c, j * 128 : (j + 1) * 128],
                            m_t[:, c, :],
                            start=(c == 0),
                            stop=(c == TC - 1),
                        )
                nc.scalar.copy(
                    zs_t[:, h * (DC // 2) : (h + 1) * (DC // 2), :],
                    zt_ps[:, :, 0:K],
                )
            zs_ts[r] = (m_t, zs_t)

        def stage2_tail(r):
            m_t, zs_t = zs_ts.pop(r)
            # counts[k] = sum_t M[t, k], transposed onto k partitions
            ct_ps = ctp.tile([1, K], f32, tag="ct")
            for c in range(TC):
                nc.tensor.matmul(
                    ct_ps[:], ones_r[:], m_t[:, c, :],
                    start=(c == 0), stop=(c == TC - 1),
                )
            ct_sb = rsp.tile([1, K], f32, tag="ct")
            nc.vector.tensor_copy(ct_sb[:], ct_ps[:])
            for c2 in range(KC):
                w = min(128, K - c2 * 128)
                tp_ps = tpp.tile([128, 16], f32, tag="tp")
                nc.tensor.transpose(
                    tp_ps[0:w, 0:1], ct_sb[0:1, c2 * 128 : c2 * 128 + w],
                    ident[0:1, 0:1],
                )
                nc.vector.tensor_copy(countsT[0:w, r, c2 : c2 + 1], tp_ps[0:w, 0:1])
            # per-row reciprocal (no cross-row barrier)
            nc.vector.tensor_scalar_max(
                countsT[:, r, :], countsT[:, r, :], 1.0
            )
            nc.vector.reciprocal(recipT[:, r, :], countsT[:, r, :])

            # logits^T[l, k] = sum_d W^T[d, l] Z^T[d, k]
            lg_ps = lgp.tile([NL, K], f32, tag="lg")
            for j in range(DC):
                nc.tensor.matmul(
                    lg_ps[:], wt[:, j, :], zs_t[:, j, :],
                    start=(j == 0), stop=(j == DC - 1),
                )
            lg_sb = rsp.tile([NL, K], f32, tag="logit")
            nc.scalar.copy(lg_sb[:], lg_ps[:])

            # tail: transpose logits, mean, +bias, sigmoid, one store
            tmp = obp.tile([128, 2, NL], f32, tag="tmp")
            for c2 in range(KC):
                w = min(128, K - c2 * 128)
                tp_ps = tpp.tile([128, 16], f32, tag="tp")
                nc.tensor.transpose(
                    tp_ps[0:w, 0:NL],
                    lg_sb[:, c2 * 128 : c2 * 128 + w],
                    ident[0:NL, 0:NL],
                )
                nc.vector.tensor_scalar(
                    tmp[0:w, c2, :], tp_ps[0:w, 0:NL], recipT[0:w, r, c2 : c2 + 1],
                    None, op0=Alu.mult,
                )
            row_out = obp.tile([128, TC, NL], f32, tag="row")
            tmp2 = obp.tile([128, 2, NL], f32, tag="tmp2")
            nc.vector.tensor_add(tmp2[:, 0, :], tmp[:, 0, :], b_bc2[:, 0, :])
            nc.vector.tensor_add(tmp2[0:96, 1, :], tmp[0:96, 1, :], b_bc2[0:96, 1, :])
            nc.scalar.activation(row_out[:, 0, :], tmp2[:, 0, :], Act.Sigmoid)
            nc.scalar.activation(row_out[0:96, 1, :], tmp2[0:96, 1, :], Act.Sigmoid)
            # slots 224..255 (tail of chunk 1) are constant sigmoid(b)
            nc.vector.tensor_copy(row_out[96:128, 1, :], sigb2[96:128, 0, :])
            nc.vector.tensor_copy(row_out[:, 2:4, :], sigb2[:])
            nc.scalar.dma_start(
                out_d[r].rearrange("(c p) l -> p c l", p=128), row_out[:]
            )

        for r in range(RPC):
            stage1(r)
            if r > 0:
                stage2_tail(r - 1)
        stage2_tail(RPC - 1)

    nc.compile()
    return nc


_NC_CACHE: dict = {}


def make_in_maps(token_features, input_mask, first_label_mask, W, b):
    x = np.ascontiguousarray(token_features, dtype=np.float32)
    im = np.ascontiguousarray(input_mask, dtype=np.int32)
    fm = np.ascontiguousarray(first_label_mask, dtype=np.int32)
    wt = np.ascontiguousarray(np.asarray(W, dtype=np.float32).T)
    bb = np.ascontiguousarray(np.asarray(b, dtype=np.float32).reshape(1, NL))
    in_maps = []
    for i in range(N_CORES):
        sl = slice(i * RPC, (i + 1) * RPC)
        in_maps.append(
            {"x": x[sl], "im": im[sl], "fm": fm[sl], "wt": wt, "b": bb}
        )
    return in_maps


def gather_out(res):
    out = np.concatenate([res.results[i]["out"] for i in range(N_CORES)], axis=0)
    return out.astype(np.float32)


def kernel(token_features, input_mask, first_label_mask, W, b):
    if "nc" not in _NC_CACHE:
        _NC_CACHE["nc"] = _build_nc()
    nc = _NC_CACHE["nc"]
    in_maps = make_in_maps(token_features, input_mask, first_label_mask, W, b)
    res = run_bass_kernel_spmd(nc, in_maps, list(range(N_CORES)))
    return gather_out(res)


if __name__ == "__main__":
    rng = np.random.default_rng(0)
    tf = rng.standard_normal((B, S, D), dtype=np.float32)
    lengths = rng.integers(16, S + 1, size=(B,))
    pos = np.arange(S)[None, :]
    im = (pos < lengths[:, None]).astype(np.int32)
    fm = ((rng.random((B, S)) < 0.4) & (im > 0)).astype(np.int32)
    fm[:, 1] = 1
    W = (rng.standard_normal((NL, D)) * 0.02).astype(np.float32)
    b = np.zeros(NL, np.float32)
    out = kernel(
        token_features=tf, input_mask=im, first_label_mask=fm, W=W, b=b
    )
    print(out.shape, out.dtype)



# revision 8
# speedup vs baseline: 1.2830x; 1.2830x over previous
"""Trainium2 Bass kernel for fused BERT-CRF-NER word_embedding + sigmoid.

Math (per batch row):
  inner[t]   = 1 <= t <= L-2          (L = valid length from contiguous mask)
  starts     = first_label_mask & inner
  wid2[t]    = cumsum(starts) * inner (1-based word id, 0 outside inner)
  wv[k]      = mean of token_features[t] over wid2[t] == k+1
  emission   = sigmoid(wv @ W.T + b)  (empty word slots -> sigmoid(b))

Restructuring for the hardware:
  1) membership matrix M[t, k] = (wid2[t] == k+1)            [128-chunk, K]
  2) Z^T[d, k]  = sum_t X[t, d] M[t, k]     (PE, X chunks stationary in the
     natural [t, d] layout -> X is never transposed)
  3) lg^T[l, k] = sum_d W^T[d, l] Z^T[d, k] + b[l]*max(cnt[k], 1)
     (bias folded in as a rank-1 matmul so (lg/cnt) = logits + b exactly,
      and empty slots come out as sigmoid(b) for free)
  4) transpose lg^T together with a stacked 1/cnt row (11-row transpose),
     then one fused ACT op per column group: sigmoid(lg * recip_scale)
  5) one 160B-per-partition output store per row (word slots p-major)

Ragged specialization: lengths vary 16..512, so the host sorts rows by
length and deals them round-robin to the 8 cores (slot j on every core
holds rows of similar length).  Per-slot token-chunk count TC[j] and
word capacity K4[j] are derived from the actual masks at runtime and
baked into the compiled program (cached per (TC, K4) tuple).  This cuts
both HBM traffic and PE work ~40% vs processing full 512-token rows.

Heavy matmuls run in bf16 (X cast fp32->bf16 during the SWDGE DMA),
accumulation in fp32 PSUM.  Sharding: pure data parallel, 8 rows/core.
"""

from contextlib import ExitStack

import numpy as np

import concourse.bass as bass
import concourse.tile as tile
from concourse import bacc, mybir
from concourse.bass_utils import run_bass_kernel_spmd

B, S, D, NL = 64, 512, 768, 10
N_CORES = 8
RPC = B // N_CORES  # batch rows (slots) per core
DC = D // 128       # feature chunks of 128

f32 = mybir.dt.float32
bf16 = mybir.dt.bfloat16
i32 = mybir.dt.int32
Alu = mybir.AluOpType
Act = mybir.ActivationFunctionType


def _plan(input_mask, first_label_mask):
    """Host-side integer metadata: row->slot assignment and per-slot caps."""
    im = np.asarray(input_mask, np.int64)
    fm = np.asarray(first_label_mask, np.int64)
    L = im.sum(1)
    pos = np.arange(S)
    inner = (im > 0) & (pos[None, :] >= 1) & (pos[None, :] <= (L - 2)[:, None])
    words = ((fm > 0) & inner).sum(1)
    order = np.argsort(-L, kind="stable")  # slot j, core i -> order[j*8+i]
    TC, K4 = [], []
    for j in range(RPC):
        rows = order[j * N_CORES : (j + 1) * N_CORES]
        TC.append(max(1, -(-int(L[rows].max()) // 128)))
        K4.append(max(4, -(-int(words[rows].max()) // 4) * 4))
    return order, tuple(TC), tuple(K4)


def _build_nc(TC, K4):
    KM = max(K4)
    assert max(TC) <= S // 128 and KM <= 256
    nc = bacc.Bacc("TRN2", target_bir_lowering=False, debug=False)
    x_d = nc.dram_tensor("x", [RPC, S, D], f32, kind="ExternalInput")
    im_d = nc.dram_tensor("im", [RPC, S], i32, kind="ExternalInput")
    fm_d = nc.dram_tensor("fm", [RPC, S], i32, kind="ExternalInput")
    wt_d = nc.dram_tensor("wt", [128, DC * NL], f32, kind="ExternalInput")
    b_d = nc.dram_tensor("b", [1, NL], f32, kind="ExternalInput")
    ck_d = nc.dram_tensor("ck", [128, KM], bf16, kind="ExternalInput")
    ci_d = nc.dram_tensor("ci", [16, 16], f32, kind="ExternalInput")
    c1_d = nc.dram_tensor("c1", [1, 128], f32, kind="ExternalInput")
    cr_d = nc.dram_tensor("cr", [128, 1], bf16, kind="ExternalInput")
    out_d = nc.dram_tensor("out", [RPC, S, NL], f32, kind="ExternalOutput")

    with tile.TileContext(nc) as tc, ExitStack() as ctx:
        const = ctx.enter_context(tc.tile_pool(name="const", bufs=1))
        xp = ctx.enter_context(tc.tile_pool(name="xp", bufs=sum(TC)))
        mp = ctx.enter_context(tc.tile_pool(name="mp", bufs=6))
        zsp = ctx.enter_context(tc.tile_pool(name="zsp", bufs=2))
        rsp = ctx.enter_context(tc.tile_pool(name="rsp", bufs=2))
        obp = ctx.enter_context(tc.tile_pool(name="obp", bufs=2))
        ztp = ctx.enter_context(
            tc.tile_pool(name="ztp", bufs=2, space=bass.MemorySpace.PSUM)
        )
        lgp = ctx.enter_context(
            tc.tile_pool(name="lgp", bufs=1, space=bass.MemorySpace.PSUM)
        )
        ctp = ctx.enter_context(
            tc.tile_pool(name="ctp", bufs=1, space=bass.MemorySpace.PSUM)
        )
        tpp = ctx.enter_context(
            tc.tile_pool(name="tpp", bufs=2, space=bass.MemorySpace.PSUM)
        )

        # ---- X chunk loads first: they pace the whole kernel ------------
        # One SWDGE cast-DMA (fp32 -> bf16) per 128-token chunk; both DRAM
        # and SBUF sides are one contiguous run per partition.  Issue order
        # == PE consumption order (slots descending by length).
        xs = {}
        for j in range(RPC):
            for c in range(TC[j]):
                x_t = xp.tile([128, D], bf16, tag="x", name=f"x{j}_{c}")
                nc.gpsimd.dma_start(x_t[:], x_d[j, c * 128 : (c + 1) * 128, :])
                xs[(j, c)] = x_t

        # ---- small loads on the HWDGE sync queue ------------------------
        im_i = const.tile([RPC, S], i32)
        nc.sync.dma_start(im_i[:], im_d[:, :])
        fm_i = const.tile([RPC, S], i32)
        nc.sync.dma_start(fm_i[:], fm_d[:, :])
        ck_t = const.tile([128, KM], bf16)  # iota 1..KM (exact in bf16)
        nc.sync.dma_start(ck_t[:], ck_d[:, :])
        ci_t = const.tile([16, 16], f32)  # identity for PE transposes
        nc.sync.dma_start(ci_t[:], ci_d[:, :])
        c1_t = const.tile([1, 128], f32)  # ones row (partition broadcast)
        nc.sync.dma_start(c1_t[:], c1_d[:, :])
        cr_t = const.tile([128, 1], bf16)  # ones column (counts matmul)
        nc.sync.dma_start(cr_t[:], cr_d[:, :])
        wt_f = const.tile([128, DC * NL], f32)  # host-permuted W^T
        nc.sync.dma_start(wt_f[:], wt_d[:, :])
        b_sb = const.tile([1, NL], f32)
        nc.sync.dma_start(b_sb[:], b_d[:, :])

        # ---- mask pipeline: 1-based word ids, all RPC rows at once ------
        # contiguous mask => inner[t] = (1 <= t <= L-2) = im[t+1] (t>=1)
        inner = const.tile([RPC, S], f32)
        nc.vector.tensor_copy(inner[:, 1 : S - 1], im_i[:, 2:S])
        nc.vector.memset(inner[:, 0:1], 0.0)
        nc.vector.memset(inner[:, S - 1 : S], 0.0)
        fmf = const.tile([RPC, S], f32)
        nc.vector.tensor_copy(fmf[:], fm_i[:])
        starts = const.tile([RPC, S], f32)
        nc.vector.tensor_mul(starts[:], fmf[:], inner[:])
        widr = const.tile([RPC, S], f32)
        nc.vector.tensor_tensor_scan(
            widr[:], starts[:], starts[:], 0.0, op0=Alu.add, op1=Alu.bypass
        )
        wid2 = const.tile([RPC, S], f32)
        nc.vector.tensor_mul(wid2[:], widr[:], inner[:])

        # casts needed later (off the mask critical path)
        b16 = const.tile([1, NL], bf16)
        nc.vector.tensor_copy(b16[:], b_sb[:])
        wt16 = const.tile([128, DC * NL], bf16)
        nc.vector.tensor_copy(wt16[:], wt_f[:])
        wt_v = wt16[:].rearrange("p (j l) -> p j l", l=NL)

        # sigmoid(b) broadcast [128, 4, NL] for the constant word region
        sigb_row = const.tile([1, NL], f32)
        nc.scalar.activation(sigb_row[:], b_sb[:], Act.Sigmoid)
        sb_ps = tpp.tile([128, 16], f32, tag="tp")
        nc.tensor.matmul(sb_ps[:, 0:NL], c1_t[0:1, :], sigb_row[0:1, :])
        sigb4 = const.tile([128, 4, NL], f32)
        for c in range(4):
            nc.scalar.copy(sigb4[:, c, :], sb_ps[:, 0:NL])

        # transpose word ids onto token partitions (chunk-major: t = c*128+p)
        TCM = max(TC)
        widT = const.tile([128, TCM, RPC], f32)
        wid_v = wid2[:].rearrange("r (c p) -> r c p", p=128)
        for c in range(TCM):
            tp_ps = tpp.tile([128, 16], f32, tag="tp")
            nc.tensor.transpose(tp_ps[:, 0:RPC], wid_v[:, c, :], ci_t[0:RPC, 0:RPC])
            nc.vector.tensor_copy(widT[:, c, :], tp_ps[:, 0:RPC])

        # ---- heavy per-slot pipeline, software-pipelined ----------------
        state = {}

        def stage1(j):
            K = K4[j]
            KP = 256 if K > 128 else 128
            m_ts = []
            for c in range(TC[j]):
                m_t = mp.tile([128, KM], bf16, tag="m", name=f"m{j}_{c}")
                nc.vector.tensor_scalar(
                    m_t[:, 0:K], ck_t[:, 0:K], widT[:, c, j : j + 1], None,
                    op0=Alu.is_equal,
                )
                m_ts.append(m_t)
            zs = zsp.tile([128, DC, KM], bf16, tag="zs", name=f"zs{j}")
            ct_ps = ctp.tile([1, KM], f32, tag="ct", name=f"ct{j}")
            # lg_sb rows 0..9 = logits (scalar, in tail), row 10 = 1/cnt
            lg_sb = rsp.tile([16, KM], f32, tag="lg", name=f"lg{j}")
            for h in range(2):
                zt = ztp.tile([128, DC // 2, KP], f32, tag="zt", name=f"zt{j}_{h}")
                # accumulation groups must be consecutive instructions:
                # keep the token-chunk loop innermost per PSUM region
                for jj in range(DC // 2):
                    dd = (h * (DC // 2) + jj) * 128
                    for c in range(TC[j]):
                        nc.tensor.matmul(
                            zt[:, jj, 0:K],
                            xs[(j, c)][:, dd : dd + 128],
                            m_ts[c][:, 0:K],
                            start=(c == 0),
                            stop=(c == TC[j] - 1),
                        )
                if h == 0:
                    for c in range(TC[j]):
                        nc.tensor.matmul(
                            ct_ps[:, 0:K], cr_t[:, 0:1], m_ts[c][:, 0:K],
                            start=(c == 0), stop=(c == TC[j] - 1),
                        )
                nc.scalar.copy(
                    zs[:, h * (DC // 2) : (h + 1) * (DC // 2), 0:K],
                    zt[:, :, 0:K],
                )
            # counts post: cnt' = max(cnt,1); 1/cnt' into lg_sb row 10;
            # bf16 cnt' for the bias rank-1 matmul
            ct_sb = rsp.tile([1, KM], f32, tag="cts", name=f"cs{j}")
            nc.vector.tensor_scalar_max(ct_sb[:, 0:K], ct_ps[:, 0:K], 1.0)
            rc_row = rsp.tile([1, KM], f32, tag="rcr", name=f"rr{j}")
            nc.vector.reciprocal(rc_row[:, 0:K], ct_sb[:, 0:K])
            # engines can't write partition 10 directly (32-aligned starts);
            # a tiny SBUF->SBUF DMA has no such restriction
            nc.sync.dma_start(lg_sb[NL : NL + 1, 0:K], rc_row[:, 0:K])
            ct16 = rsp.tile([1, KM], bf16, tag="ct16", name=f"c6{j}")
            nc.vector.tensor_copy(ct16[:, 0:K], ct_sb[:, 0:K])
            state[j] = (zs, lg_sb, ct16)

        def stage2_tail(j):
            K = K4[j]
            W = K // 4
            zs, lg_sb, ct16 = state.pop(j)
            # lg^T[l,k] = sum_d W^T[d,l] Z^T[d,k] + b[l]*cnt'[k]
            lg_ps = lgp.tile([NL, KM], f32, tag="lgp", name=f"lp{j}")
            for jj in range(DC):
                nc.tensor.matmul(
                    lg_ps[:, 0:K], wt_v[:, jj, :], zs[:, jj, 0:K],
                    start=(jj == 0), stop=False,
                )
            nc.tensor.matmul(
                lg_ps[:, 0:K], b16[0:1, :], ct16[0:1, 0:K],
                start=False, stop=True,
            )
            nc.scalar.copy(lg_sb[0:NL, 0:K], lg_ps[:, 0:K])

            # 11-row transposes (logits + recip), fused scale+sigmoid, store
            row_out = obp.tile([128, 4, NL], f32, tag="row", name=f"ro{j}")
            recipT = obp.tile([128, 4], f32, tag="rT", name=f"rt{j}")
            # fill everything with sigmoid(b) first; computed word slots
            # [0:W] are overwritten below (partition starts must be 32-aligned
            # so a [W:128] fill is not expressible directly)
            nc.vector.tensor_copy(row_out[:], sigb4[:])
            lg_v = lg_sb[:].rearrange("l (p c) -> l p c", c=4)
            for c in range(4):
                tp_ps = tpp.tile([128, 16], f32, tag="tp")
                nc.tensor.transpose(
                    tp_ps[0:W, 0 : NL + 1], lg_v[0 : NL + 1, 0:W, c],
                    ci_t[0 : NL + 1, 0 : NL + 1],
                )
                nc.vector.tensor_copy(recipT[0:W, c : c + 1], tp_ps[0:W, NL : NL + 1])
                nc.scalar.activation(
                    row_out[0:W, c, :], tp_ps[0:W, 0:NL], Act.Sigmoid,
                    scale=recipT[0:W, c : c + 1],
                )
            nc.scalar.dma_start(
                out_d[j].rearrange("(p c) l -> p c l", c=4), row_out[:]
            )

        for j in range(RPC):
            stage1(j)
            if j > 0:
                stage2_tail(j - 1)
        stage2_tail(RPC - 1)

    nc.compile()
    return nc


_NC_CACHE: dict = {}


def _prepare(input_mask, first_label_mask):
    order, TC, K4 = _plan(input_mask, first_label_mask)
    if _NC_CACHE.get("key") != (TC, K4):
        _NC_CACHE["nc"] = _build_nc(TC, K4)
        _NC_CACHE["key"] = (TC, K4)
    _NC_CACHE["order"] = order
    _NC_CACHE["KM"] = max(K4)
    return _NC_CACHE["nc"]


def make_in_maps(token_features, input_mask, first_label_mask, W, b):
    _prepare(input_mask, first_label_mask)
    order, KM = _NC_CACHE["order"], _NC_CACHE["KM"]
    np_bf16 = mybir.dt.np(bf16)
    x = np.asarray(token_features, dtype=np.float32)
    im = np.asarray(input_mask, dtype=np.int32)
    fm = np.asarray(first_label_mask, dtype=np.int32)
    # host-permuted W^T: wt[p, j*NL+l] = W[l, j*128+p]
    wt = np.ascontiguousarray(
        np.asarray(W, dtype=np.float32).T.reshape(DC, 128, NL)
        .transpose(1, 0, 2).reshape(128, DC * NL)
    )
    bb = np.ascontiguousarray(np.asarray(b, dtype=np.float32).reshape(1, NL))
    ck = np.ascontiguousarray(
        np.broadcast_to(np.arange(1, KM + 1, dtype=np.float32), (128, KM))
    ).astype(np_bf16)
    ci = np.eye(16, dtype=np.float32)
    c1 = np.ones((1, 128), np.float32)
    cr = np.ones((128, 1), np_bf16)
    in_maps = []
    for i in range(N_CORES):
        rows = [order[j * N_CORES + i] for j in range(RPC)]
        in_maps.append(
            {
                "x": np.ascontiguousarray(x[rows]),
                "im": np.ascontiguousarray(im[rows]),
                "fm": np.ascontiguousarray(fm[rows]),
                "wt": wt, "b": bb, "ck": ck, "ci": ci, "c1": c1, "cr": cr,
            }
        )
    return in_maps


def gather_out(res):
    order = _NC_CACHE["order"]
    out = np.empty((B, S, NL), np.float32)
    for i in range(N_CORES):
        o = res.results[i]["out"]
        for j in range(RPC):
            out[order[j * N_CORES + i]] = o[j]
    return out


def kernel(token_features, input_mask, first_label_mask, W, b):
    nc = _prepare(input_mask, first_label_mask)
    in_maps = make_in_maps(token_features, input_mask, first_label_mask, W, b)
    res = run_bass_kernel_spmd(nc, in_maps, list(range(N_CORES)))
    return gather_out(res)


if __name__ == "__main__":
    rng = np.random.default_rng(0)
    tf = rng.standard_normal((B, S, D), dtype=np.float32)
    lengths = rng.integers(16, S + 1, size=(B,))
    pos = np.arange(S)[None, :]
    im = (pos < lengths[:, None]).astype(np.int32)
    fm = ((rng.random((B, S)) < 0.4) & (im > 0)).astype(np.int32)
    fm[:, 1] = 1
    W = (rng.standard_normal((NL, D)) * 0.02).astype(np.float32)
    b = np.zeros(NL, np.float32)
    out = kernel(
        token_features=tf, input_mask=im, first_label_mask=fm, W=W, b=b
    )
    print(out.shape, out.dtype)


# revision 14
# speedup vs baseline: 1.3765x; 1.0728x over previous
"""Trainium2 Bass kernel for fused BERT-CRF-NER word_embedding + sigmoid.

Math (per batch row):
  inner[t]   = 1 <= t <= L-2          (L = valid length from contiguous mask)
  starts     = first_label_mask & inner
  wid2[t]    = cumsum(starts) * inner (1-based word id, 0 outside inner)
  wv[k]      = mean of token_features[t] over wid2[t] == k+1
  emission   = sigmoid(wv @ W.T + b)  (empty word slots -> sigmoid(b))

Restructuring for the hardware:
  1) membership matrix M[t, k] = (wid2[t] == k+1)            [128-chunk, K]
  2) Z^T[d, k]  = sum_t X[t, d] M[t, k]     (PE, X chunks stationary in the
     natural [t, d] layout -> X is never transposed)
  3) lg^T[l, k] = sum_d W^T[d, l] Z^T[d, k] + b[l]*max(cnt[k], 1)
     (bias folded in as a rank-1 matmul so (lg/cnt) = logits + b exactly,
      and empty slots come out as sigmoid(b) for free)
  4) transpose lg^T together with a stacked 1/cnt row (11-row bf16
     transpose), then one fused ACT op per column group:
     sigmoid(lg * recip_scale)
  5) one 160B-per-partition output store per row (word slots p-major)

Ragged specialization: lengths vary 16..512, so the host sorts rows by
length and deals them round-robin to the 8 cores (slot j on every core
holds rows of similar length).  Per-slot token-chunk count TC[j] and
word capacity K4[j] are derived from the actual masks at runtime and
baked into the compiled program (cached per (TC, K4) tuple).  This cuts
both HBM traffic and PE work ~40% vs processing full 512-token rows.

Heavy matmuls run in bf16 (X cast fp32->bf16 during the SWDGE DMA),
accumulation in fp32 PSUM.  All PE transposes use bf16 stationaries
(fp32 pays a double-pass LDWEIGHTS).  Sharding: data parallel, 8
rows/core.
"""

from contextlib import ExitStack

import numpy as np

import concourse.bass as bass
import concourse.tile as tile
from concourse import bacc, mybir
from concourse.bass_utils import run_bass_kernel_spmd

B, S, D, NL = 64, 512, 768, 10
N_CORES = 8
RPC = B // N_CORES  # batch rows (slots) per core
DC = D // 128       # feature chunks of 128

f32 = mybir.dt.float32
bf16 = mybir.dt.bfloat16
i32 = mybir.dt.int32
Alu = mybir.AluOpType
Act = mybir.ActivationFunctionType


def _plan(input_mask, first_label_mask):
    """Host-side integer metadata: row->slot assignment and per-slot caps."""
    im = np.asarray(input_mask, np.int64)
    fm = np.asarray(first_label_mask, np.int64)
    L = im.sum(1)
    pos = np.arange(S)
    inner = (im > 0) & (pos[None, :] >= 1) & (pos[None, :] <= (L - 2)[:, None])
    words = ((fm > 0) & inner).sum(1)
    order = np.argsort(-L, kind="stable")  # slot j, core i -> order[j*8+i]
    TC, K4 = [], []
    for j in range(RPC):
        rows = order[j * N_CORES : (j + 1) * N_CORES]
        TC.append(max(1, -(-int(L[rows].max()) // 128)))
        K4.append(max(4, -(-int(words[rows].max()) // 4) * 4))
    return order, tuple(TC), tuple(K4)


def _build_nc(TC, K4):
    KM = max(K4)
    assert max(TC) <= S // 128 and KM <= 256
    nc = bacc.Bacc("TRN2", target_bir_lowering=False, debug=False)
    x_d = nc.dram_tensor("x", [RPC, S, D], f32, kind="ExternalInput")
    msk_d = nc.dram_tensor("msk", [RPC, 2 * S], i32, kind="ExternalInput")
    wt_d = nc.dram_tensor("wt", [128, DC * NL], f32, kind="ExternalInput")
    b_d = nc.dram_tensor("b", [1, NL], f32, kind="ExternalInput")
    ck_d = nc.dram_tensor("ck", [128, KM], bf16, kind="ExternalInput")
    ci_d = nc.dram_tensor("ci", [16, 16], bf16, kind="ExternalInput")
    c1_d = nc.dram_tensor("c1", [1, 128], f32, kind="ExternalInput")
    cr_d = nc.dram_tensor("cr", [128, 1], bf16, kind="ExternalInput")
    out_d = nc.dram_tensor("out", [RPC, S, NL], f32, kind="ExternalOutput")

    with tile.TileContext(nc) as tc, ExitStack() as ctx:
        const = ctx.enter_context(tc.tile_pool(name="const", bufs=1))
        xp = ctx.enter_context(tc.tile_pool(name="xp", bufs=sum(TC)))
        mp = ctx.enter_context(tc.tile_pool(name="mp", bufs=6))
        zsp = ctx.enter_context(tc.tile_pool(name="zsp", bufs=2))
        rsp = ctx.enter_context(tc.tile_pool(name="rsp", bufs=2))
        obp = ctx.enter_context(tc.tile_pool(name="obp", bufs=2))
        ztp = ctx.enter_context(
            tc.tile_pool(name="ztp", bufs=2, space=bass.MemorySpace.PSUM)
        )
        lgp = ctx.enter_context(
            tc.tile_pool(name="lgp", bufs=1, space=bass.MemorySpace.PSUM)
        )
        ctp = ctx.enter_context(
            tc.tile_pool(name="ctp", bufs=1, space=bass.MemorySpace.PSUM)
        )
        tpp = ctx.enter_context(
            tc.tile_pool(name="tpp", bufs=2, space=bass.MemorySpace.PSUM)
        )

        # ---- X chunk loads first: they pace the whole kernel ------------
        # One SWDGE cast-DMA (fp32 -> bf16) per 128-token chunk; both DRAM
        # and SBUF sides are one contiguous run per partition.  Issue order
        # == PE consumption order (slots descending by length).
        xs = {}
        for j in range(RPC):
            for c in range(TC[j]):
                x_t = xp.tile([128, D], bf16, tag="x", name=f"x{j}_{c}")
                nc.gpsimd.dma_start(x_t[:], x_d[j, c * 128 : (c + 1) * 128, :])
                xs[(j, c)] = x_t

        # ---- small loads on the HWDGE sync queue (mask-chain deps first)
        msk_i = const.tile([RPC, 2 * S], i32)
        nc.sync.dma_start(msk_i[:], msk_d[:, :])
        ci_t = const.tile([16, 16], bf16)  # identity for PE transposes
        nc.sync.dma_start(ci_t[:], ci_d[:, :])
        ck_t = const.tile([128, KM], bf16)  # iota 1..KM (exact in bf16)
        nc.sync.dma_start(ck_t[:], ck_d[:, :])
        cr_t = const.tile([128, 1], bf16)  # ones column (counts matmul)
        nc.sync.dma_start(cr_t[:], cr_d[:, :])
        c1_t = const.tile([1, 128], f32)  # ones row (partition broadcast)
        nc.sync.dma_start(c1_t[:], c1_d[:, :])
        wt_f = const.tile([128, DC * NL], f32)  # host-permuted W^T
        nc.sync.dma_start(wt_f[:], wt_d[:, :])
        b_sb = const.tile([1, NL], f32)
        nc.sync.dma_start(b_sb[:], b_d[:, :])

        # ---- mask pipeline: 1-based word ids, all RPC rows at once ------
        # contiguous mask => inner[t] = (1 <= t <= L-2) = im[t+1] (t>=1)
        im_v = msk_i[:, 0:S]
        fm_v = msk_i[:, S : 2 * S]
        inner = const.tile([RPC, S], f32)
        nc.vector.tensor_copy(inner[:, 1 : S - 1], im_v[:, 2:S])
        nc.vector.memset(inner[:, 0:1], 0.0)
        nc.vector.memset(inner[:, S - 1 : S], 0.0)
        starts = const.tile([RPC, S], f32)
        nc.vector.tensor_mul(starts[:], fm_v, inner[:])
        widr = const.tile([RPC, S], f32)
        nc.vector.tensor_tensor_scan(
            widr[:], starts[:], starts[:], 0.0, op0=Alu.add, op1=Alu.bypass
        )
        wid2 = const.tile([RPC, S], bf16)
        nc.vector.tensor_mul(wid2[:], widr[:], inner[:])

        # transpose word ids onto token partitions (chunk-major: t = c*128+p)
        TCM = max(TC)
        widT = const.tile([128, TCM, RPC], f32)  # is_equal scalar must be f32
        wid_v = wid2[:].rearrange("r (c p) -> r c p", p=128)
        for c in range(TCM):
            tp_ps = tpp.tile([128, 16], bf16, tag="tp")
            nc.tensor.transpose(tp_ps[:, 0:RPC], wid_v[:, c, :], ci_t[0:RPC, 0:RPC])
            nc.vector.tensor_copy(widT[:, c, :], tp_ps[:, 0:RPC])

        # late consts (emitted after slot 0 below): bias + sigmoid(b) bcast
        b16 = const.tile([1, NL], bf16)
        wt16 = const.tile([128, DC * NL], bf16)
        wt_v = wt16[:].rearrange("p (j l) -> p j l", l=NL)
        sigb_row = const.tile([1, NL], f32)
        sigb4 = const.tile([128, 4, NL], f32)

        def setup_late_consts():
            nc.vector.tensor_copy(b16[:], b_sb[:])
            nc.vector.tensor_copy(wt16[:], wt_f[:])
            nc.scalar.activation(sigb_row[:], b_sb[:], Act.Sigmoid)
            sb_ps = lgp.tile([128, 16], f32, tag="lgp")
            nc.tensor.matmul(sb_ps[:, 0:NL], c1_t[0:1, :], sigb_row[0:1, :])
            for c in range(4):
                nc.scalar.copy(sigb4[:, c, :], sb_ps[:, 0:NL])

        # ---- heavy per-slot pipeline, software-pipelined ----------------
        state = {}

        def stage1(j):
            K = K4[j]
            KP = 256 if K > 128 else 128
            m_ts = []
            for c in range(TC[j]):
                m_t = mp.tile([128, KM], bf16, tag="m", name=f"m{j}_{c}")
                nc.vector.tensor_scalar(
                    m_t[:, 0:K], ck_t[:, 0:K], widT[:, c, j : j + 1], None,
                    op0=Alu.is_equal,
                )
                m_ts.append(m_t)
            zs = zsp.tile([128, DC, KM], bf16, tag="zs", name=f"zs{j}")
            ct_ps = ctp.tile([1, KM], f32, tag="ct", name=f"ct{j}")
            # lg_sb rows 0..9 = logits (scalar, in tail), row 10 = 1/cnt
            lg_sb = rsp.tile([16, KM], bf16, tag="lg", name=f"lg{j}")
            for h in range(2):
                zt = ztp.tile([128, DC // 2, KP], f32, tag="zt", name=f"zt{j}_{h}")
                # accumulation groups must be consecutive instructions:
                # keep the token-chunk loop innermost per PSUM region
                for jj in range(DC // 2):
                    dd = (h * (DC // 2) + jj) * 128
                    for c in range(TC[j]):
                        nc.tensor.matmul(
                            zt[:, jj, 0:K],
                            xs[(j, c)][:, dd : dd + 128],
                            m_ts[c][:, 0:K],
                            start=(c == 0),
                            stop=(c == TC[j] - 1),
                        )
                if h == 0:
                    for c in range(TC[j]):
                        nc.tensor.matmul(
                            ct_ps[:, 0:K], cr_t[:, 0:1], m_ts[c][:, 0:K],
                            start=(c == 0), stop=(c == TC[j] - 1),
                        )
                dst = zs[:, h * (DC // 2) : (h + 1) * (DC // 2), 0:K]
                if h == 0:
                    nc.scalar.copy(dst, zt[:, :, 0:K])
                else:
                    nc.vector.tensor_copy(dst, zt[:, :, 0:K])
            # counts post: cnt' = max(cnt,1); 1/cnt' -> lg_sb row 10 (via a
            # tiny SBUF->SBUF DMA: engines can't write partition 10 directly,
            # partition starts must be 32-aligned); bf16 cnt' for the bias MM
            ct_sb = rsp.tile([1, KM], f32, tag="cts", name=f"cs{j}")
            nc.vector.tensor_scalar_max(ct_sb[:, 0:K], ct_ps[:, 0:K], 1.0)
            rc16 = rsp.tile([1, KM], bf16, tag="rcr", name=f"rr{j}")
            with nc.allow_low_precision(reason="bf16 1/cnt, tol 2e-2"):
                nc.vector.reciprocal(rc16[:, 0:K], ct_sb[:, 0:K])
            nc.sync.dma_start(lg_sb[NL : NL + 1, 0:K], rc16[:, 0:K])
            ct16 = rsp.tile([1, KM], bf16, tag="ct16", name=f"c6{j}")
            nc.vector.tensor_copy(ct16[:, 0:K], ct_sb[:, 0:K])
            state[j] = (zs, lg_sb, ct16)

        def stage2_tail(j):
            K = K4[j]
            W = K // 4
            zs, lg_sb, ct16 = state.pop(j)
            # lg^T[l,k] = sum_d W^T[d,l] Z^T[d,k] + b[l]*cnt'[k]
            lg_ps = lgp.tile([NL, KM], f32, tag="lgp", name=f"lp{j}")
            for jj in range(DC):
                nc.tensor.matmul(
                    lg_ps[:, 0:K], wt_v[:, jj, :], zs[:, jj, 0:K],
                    start=(jj == 0), stop=False,
                )
            nc.tensor.matmul(
                lg_ps[:, 0:K], b16[0:1, :], ct16[0:1, 0:K],
                start=False, stop=True,
            )
            nc.scalar.copy(lg_sb[0:NL, 0:K], lg_ps[:, 0:K])

            # 11-row transposes (logits + recip), fused scale+sigmoid, store
            row_out = obp.tile([128, 4, NL], f32, tag="row", name=f"ro{j}")
            recipT = obp.tile([128, 4], f32, tag="rT", name=f"rt{j}")
            # fill everything with sigmoid(b) first; computed word slots
            # [0:W] are overwritten below
            nc.vector.tensor_copy(row_out[:], sigb4[:])
            lg_v = lg_sb[:].rearrange("l (p c) -> l p c", c=4)
            for c in range(4):
                tp_ps = tpp.tile([128, 16], bf16, tag="tp")
                nc.tensor.transpose(
                    tp_ps[0:W, 0 : NL + 1], lg_v[0 : NL + 1, 0:W, c],
                    ci_t[0 : NL + 1, 0 : NL + 1],
                )
                nc.vector.tensor_copy(recipT[0:W, c : c + 1], tp_ps[0:W, NL : NL + 1])
                nc.scalar.activation(
                    row_out[0:W, c, :], tp_ps[0:W, 0:NL], Act.Sigmoid,
                    scale=recipT[0:W, c : c + 1],
                )
            nc.scalar.dma_start(
                out_d[j].rearrange("(p c) l -> p c l", c=4), row_out[:]
            )

        for j in range(RPC):
            stage1(j)
            if j == 0:
                setup_late_consts()
            if j > 0:
                stage2_tail(j - 1)
        stage2_tail(RPC - 1)

    nc.compile()
    return nc


_NC_CACHE: dict = {}


def _prepare(input_mask, first_label_mask):
    order, TC, K4 = _plan(input_mask, first_label_mask)
    if _NC_CACHE.get("key") != (TC, K4):
        _NC_CACHE["nc"] = _build_nc(TC, K4)
        _NC_CACHE["key"] = (TC, K4)
    _NC_CACHE["order"] = order
    _NC_CACHE["KM"] = max(K4)
    return _NC_CACHE["nc"]


def make_in_maps(token_features, input_mask, first_label_mask, W, b):
    _prepare(input_mask, first_label_mask)
    order, KM = _NC_CACHE["order"], _NC_CACHE["KM"]
    np_bf16 = mybir.dt.np(bf16)
    x = np.asarray(token_features, dtype=np.float32)
    im = np.asarray(input_mask, dtype=np.int32)
    fm = np.asarray(first_label_mask, dtype=np.int32)
    msk = np.concatenate([im, fm], axis=1)  # [B, 2S]
    # host-permuted W^T: wt[p, j*NL+l] = W[l, j*128+p]
    wt = np.ascontiguousarray(
        np.asarray(W, dtype=np.float32).T.reshape(DC, 128, NL)
        .transpose(1, 0, 2).reshape(128, DC * NL)
    )
    bb = np.ascontiguousarray(np.asarray(b, dtype=np.float32).reshape(1, NL))
    ck = np.ascontiguousarray(
        np.broadcast_to(np.arange(1, KM + 1, dtype=np.float32), (128, KM))
    ).astype(np_bf16)
    ci = np.eye(16, dtype=np.float32).astype(np_bf16)
    c1 = np.ones((1, 128), np.float32)
    cr = np.ones((128, 1), np_bf16)
    in_maps = []
    for i in range(N_CORES):
        rows = [order[j * N_CORES + i] for j in range(RPC)]
        in_maps.append(
            {
                "x": np.ascontiguousarray(x[rows]),
                "msk": np.ascontiguousarray(msk[rows]),
                "wt": wt, "b": bb, "ck": ck, "ci": ci, "c1": c1, "cr": cr,
            }
        )
    return in_maps


def gather_out(res):
    order = _NC_CACHE["order"]
    out = np.empty((B, S, NL), np.float32)
    for i in range(N_CORES):
        o = res.results[i]["out"]
        for j in range(RPC):
            out[order[j * N_CORES + i]] = o[j]
    return out


def kernel(token_features, input_mask, first_label_mask, W, b):
    nc = _prepare(input_mask, first_label_mask)
    in_maps = make_in_maps(token_features, input_mask, first_label_mask, W, b)
    res = run_bass_kernel_spmd(nc, in_maps, list(range(N_CORES)))
    return gather_out(res)


if __name__ == "__main__":
    rng = np.random.default_rng(0)
    tf = rng.standard_normal((B, S, D), dtype=np.float32)
    lengths = rng.integers(16, S + 1, size=(B,))
    pos = np.arange(S)[None, :]
    im = (pos < lengths[:, None]).astype(np.int32)
    fm = ((rng.random((B, S)) < 0.4) & (im > 0)).astype(np.int32)
    fm[:, 1] = 1
    W = (rng.standard_normal((NL, D)) * 0.02).astype(np.float32)
    b = np.zeros(NL, np.float32)
    out = kernel(
        token_features=tf, input_mask=im, first_label_mask=fm, W=W, b=b
    )
    print(out.shape, out.dtype)
